# revision 28
# baseline (speedup 1.0000x reference)
"""Causal ALiBi attention (GQA) on 8 Trainium2 NeuronCores.

Sharding: 2 (batch) x 4 (head groups). Core (b, g) computes batch b, query
heads 4g..4g+3 and kv head g, producing a partial output (its heads'
contribution to the out-projection). Host sums the 4 partials per batch and
adds bo.

Per-core kernel (matmuls bf16, fp32 PSUM accumulation):
  - input DMA is chunked along the contraction dim and K/V projections
    consume chunks as they land, so the PE never waits for the full 9 MB
    x^T transfer. V is produced directly token-major (lhsT = x^T chunk).
  - scores are computed TRANSPOSED: S^T[k,q] = sum_d kT[d,k] qT[d,q], so
    softmax probabilities come out of the PE already k-major and feed the
    AV matmul with no transposes at all.
  - softmax without max-reduction: P''[k,q] = e^{S-12} * M_d[k,q] where
    M_d[k,q] = e^{slope*(k-q-128d)} (d = qi-kt tile diagonal index,
    triangular-masked at d=0) is an exact precomputed bf16 factor. exp is
    one wide ACT instruction per 512-col chunk (scalar bias -12); the
    alibi+mask factor is one batched DVE multiply per chunk. Entries that
    underflow bf16 have true softmax weight < e^-75 of their row max.
  - AV rhs is [V | 1]-augmented, so PSUM column 128 accumulates the
    denominator l_q for free; normalization rides the PSUM->SBUF copy.
  - attention out is PE-transposed to [dh, q]; out-projection contracts
    feature chunks with Wo^T; partial out rows DMA to HBM as bf16.
  - emission interleaves Q-projection chunks, attention tiles and the
    out-projection so PE keeps running while ACT does exp and DVE does
    the mask multiplies.
"""

import numpy as np
import ml_dtypes

import concourse.bass as bass
import concourse.tile as tile
from concourse import bacc, mybir
from concourse.bass_utils import run_bass_kernel_spmd
from concourse.masks import make_identity

B, S, D = 2, 2048, 2048
H, KV, DH = 16, 4, 128
SCALE = 1.0 / np.sqrt(DH)
NCORES = 8
NG = 4            # head groups (= kv heads); one per core within a batch
HPG = H // NG     # query heads per group (4)
FPG = HPG * DH    # features per group (512)
P = 128           # partitions
QT = S // P       # q tiles (16)
KCH = (D + P) // P  # contraction chunks incl. bias ones-row chunk (17)
VW = P + 4        # padded AV psum width (129 used)
EXP_SHIFT = 12.0  # constant exponent headroom margin

_BF16 = ml_dtypes.bfloat16
_COMPILED = {}    # (causal, has_bias) -> compiled Bacc program
LAST_RUN = {}     # timing info from the most recent run


def _build(causal: bool, nkch: int = KCH):
    nc = bacc.Bacc("TRN2", target_bir_lowering=False, debug=False,
                   num_devices=NCORES)
    dt = mybir.dt
    KA = KCH * P

    xTa = nc.dram_tensor("xTa", [KA, S], dt.bfloat16, kind="ExternalInput")
    wqTa = nc.dram_tensor("wqTa", [KA, FPG], dt.bfloat16, kind="ExternalInput")
    # wk/wv packed partition-major: wkP[p, c*DH+f] = Wk^T[c*P+p, f]
    wkP = nc.dram_tensor("wkP", [P, KCH * DH], dt.bfloat16,
                         kind="ExternalInput")
    wvP = nc.dram_tensor("wvP", [P, KCH * DH], dt.bfloat16,
                         kind="ExternalInput")
    woT = nc.dram_tensor("woT", [FPG, D], dt.bfloat16, kind="ExternalInput")
    if causal:
        # mmf[k, m, h, q]: alibi factor for tile diagonal d = QT-1-m
        mmf = nc.dram_tensor("mmf", [P, QT, HPG, P], dt.bfloat16,
                             kind="ExternalInput")
    else:
        # evec[k, t, h]: e^{slope*(k_global - (S-1))} folded into V per head
        evec = nc.dram_tensor("evec", [P, QT, HPG], dt.float32,
                              kind="ExternalInput")
    out = nc.dram_tensor("out", [S, D], dt.bfloat16, kind="ExternalOutput")

    with tile.TileContext(nc) as tc:
        with tc.tile_pool(name="persist", bufs=1) as persist:
            # ---- persistent SBUF tiles ----
            ident = persist.tile([P, P], dt.bfloat16)
            make_identity(nc, ident[:])
            bshift = persist.tile([P, 1], dt.float32)
            nc.vector.memset(bshift[:], -EXP_SHIFT)
            # kT/vsb split into independent tiles so their PSUM->SBUF fills
            # can run on different engines without same-tile write chaining
            kTs = [persist.tile([P, S // 2], dt.bfloat16, name=f"kT{i}")
                   for i in range(2)]
            nv = 1 if causal else HPG
            vsbs = [persist.tile([P, nv, 4, VW], dt.bfloat16, name=f"vsb{i}")
                    for i in range(4)]

            def kT(kt):
                # [P, P] slice for k-tile kt
                return kTs[kt // 8][:, (kt % 8) * P:(kt % 8 + 1) * P]

            def vsl(h, kt):
                return vsbs[kt // 4][:, 0 if causal else h, kt % 4, 0:P + 1]
            qT = persist.tile([P, HPG, S], dt.bfloat16)
            xTa_sb = persist.tile([P, nkch, S], dt.bfloat16)
            wq_sb = persist.tile([P, nkch, FPG], dt.bfloat16)
            wk_sb = persist.tile([P, nkch, DH], dt.bfloat16)
            wv_sb = persist.tile([P, nkch, DH], dt.bfloat16)
            wo_sb = persist.tile([P, FPG // P, D], dt.bfloat16)
            if causal:
                mm_sb = persist.tile([P, QT, HPG, P], dt.bfloat16)
            else:
                ev_sb = persist.tile([P, QT, HPG], dt.float32)

            # ---- DMA issue order: phase-1-critical first; chunk-0 weights
            # land in ~200ns so the first K/V matmuls only wait on x chunk 0
            nc.sync.dma_start(wk_sb[:, 0], wkP[:, 0:DH])
            nc.sync.dma_start(wv_sb[:, 0], wvP[:, 0:DH])
            nc.sync.dma_start(xTa_sb[:, 0], xTa[0:P, :])
            nc.sync.dma_start(wk_sb[:, 1:nkch].rearrange("p c f -> p (c f)"),
                              wkP[:, DH:nkch * DH])
            nc.sync.dma_start(wv_sb[:, 1:nkch].rearrange("p c f -> p (c f)"),
                              wvP[:, DH:nkch * DH])
            for c in range(1, nkch):
                nc.sync.dma_start(xTa_sb[:, c], xTa[c * P:(c + 1) * P, :])
            if causal:
                nc.sync.dma_start(mm_sb[:, 12:16], mmf[:, 12:16])
            else:
                nc.sync.dma_start(ev_sb[:], evec[:])
            for c in range(nkch):
                nc.sync.dma_start(wq_sb[:, c], wqTa[c * P:(c + 1) * P, :])
            if causal:
                nc.sync.dma_start(mm_sb[:, 8:12], mmf[:, 8:12])
            for f in range(FPG // P):
                nc.sync.dma_start(wo_sb[:, f], woT[f * P:(f + 1) * P, :])
            if causal:
                nc.sync.dma_start(mm_sb[:, 4:8], mmf[:, 4:8])
                nc.sync.dma_start(mm_sb[:, 0:4], mmf[:, 0:4])

            # ---- phase 1: K/V projections, chunk-outer (overlaps DMA) ----
            # PSUM allows only ONE open multi-instruction accumulation group
            # per bank (a later start=True zeroes the whole bank), so each
            # V-tile accumulator owns a bank and V runs in 4 sweeps
            with tc.tile_pool(name="psum1", bufs=1, space="PSUM") as psum1:
                psKs = [psum1.tile([P, 2, 512], dt.float32, tag=f"pk{i}",
                                   bufs=1, name=f"psK{i}") for i in range(2)]
                for j in range(4):
                    nc.vector.memset(vsbs[j][:, :, :, P], 1.0)

                def vsweep(s, with_k):
                    pvs = [psum1.tile([P, P], dt.float32, tag=f"pv{j}",
                                      bufs=1, name=f"psV{s}_{j}")
                           for j in range(4)]
                    for c in range(nkch):
                        st, sp = (c == 0), (c == nkch - 1)
                        if with_k:
                            for t4 in range(4):
                                nc.tensor.matmul(
                                    psKs[t4 // 2][:, t4 % 2], wk_sb[:, c],
                                    xTa_sb[:, c, t4 * 512:(t4 + 1) * 512],
                                    start=st, stop=sp)
                        for j in range(4):
                            t = s * 4 + j
                            nc.tensor.matmul(pvs[j],
                                             xTa_sb[:, c, t * P:(t + 1) * P],
                                             wv_sb[:, c], start=st, stop=sp)
                    if with_k:
                        for t4 in range(4):
                            if t4 < 2:
                                nc.vector.tensor_copy(
                                    kTs[0][:, t4 * 512:(t4 + 1) * 512],
                                    psKs[0][:, t4])
                            else:
                                nc.scalar.copy(
                                    kTs[1][:, (t4 % 2) * 512:
                                           (t4 % 2 + 1) * 512],
                                    psKs[1][:, t4 % 2])
                    dst = vsbs[s]
                    eng = (nc.scalar, nc.vector)[s % 2]
                    for j in range(4):
                        if causal:
                            if eng is nc.scalar:
                                nc.scalar.copy(dst[:, 0, j, 0:P], pvs[j])
                            else:
                                nc.vector.tensor_copy(dst[:, 0, j, 0:P],
                                                      pvs[j])
                        else:
                            t = s * 4 + j
                            for h in range(HPG):
                                nc.vector.tensor_scalar_mul(
                                    dst[:, h, j, 0:P], pvs[j],
                                    ev_sb[:, t, h:h + 1])
                                nc.vector.tensor_copy(dst[:, h, j, P:P + 1],
                                                      ev_sb[:, t, h:h + 1])

                for s in range(4):
                    vsweep(s, with_k=(s == 0))

            # ---- phase 2+3: Q-proj / attention / out-proj interleaved ----
            with (
                tc.tile_pool(name="psum", bufs=1, space="PSUM") as psum,
                tc.tile_pool(name="work", bufs=1) as work,
                tc.tile_pool(name="small", bufs=4) as small,
            ):
                def qproj(tq, h):
                    ps = psum.tile([P, 512], dt.float32, tag="q", bufs=3)
                    for c in range(nkch):
                        nc.tensor.matmul(
                            ps[:], wq_sb[:, c, h * P:(h + 1) * P],
                            xTa_sb[:, c, tq * 512:(tq + 1) * 512],
                            start=(c == 0), stop=(c == nkch - 1))
                    nc.scalar.copy(qT[:, h, tq * 512:(tq + 1) * 512], ps[:])

                def attention(qi):
                    nkt = qi + 1 if causal else QT
                    nch = (nkt + 3) // 4
                    aq = work.tile([P, HPG, P], dt.bfloat16, tag="aq", bufs=4)
                    pex = [[None] * nch for _ in range(HPG)]

                    def qk(h):
                        for c in range(nch):
                            w = min(4, nkt - c * 4)
                            pexp = work.tile([P, 512], dt.bfloat16, tag="pex",
                                             bufs=12)
                            pex[h][c] = pexp
                            ss = psum.tile([P, 512], dt.float32, tag="sc",
                                           bufs=2)
                            for j in range(w):
                                nc.tensor.matmul(
                                    ss[:, j * P:(j + 1) * P],
                                    kT(c * 4 + j),
                                    qT[:, h, qi * P:(qi + 1) * P],
                                    start=True, stop=True)
                            nc.scalar.activation(
                                pexp[:, 0:w * P], ss[:, 0:w * P],
                                mybir.ActivationFunctionType.Exp,
                                bias=bshift[:], scale=1.0)
                            if causal:
                                seg = pexp[:, 0:w * P].rearrange(
                                    "p (n q) -> p n q", q=P)
                                m0 = QT - 1 - qi + c * 4
                                nc.vector.tensor_mul(seg, seg,
                                                     mm_sb[:, m0:m0 + w, h])

                    anorms = [None] * HPG

                    def av(h):
                        avt = psum.tile([P, VW], dt.float32, tag="av", bufs=2)
                        vh = 0 if causal else h
                        for kt in range(nkt):
                            nc.tensor.matmul(
                                avt[:, 0:P + 1],
                                pex[h][kt // 4][:, (kt % 4) * P:
                                                (kt % 4) * P + P],
                                vsl(vh, kt),
                                start=(kt == 0), stop=(kt == nkt - 1))
                        rec = small.tile([P, 1], dt.float32, tag="rec")
                        nc.vector.reciprocal(rec[:], avt[:, P:P + 1])
                        anorm = small.tile([P, P], dt.bfloat16, tag="an",
                                           bufs=8)
                        nc.vector.tensor_scalar_mul(anorm[:], avt[:, 0:P],
                                                    rec[:])
                        anorms[h] = anorm

                    def finalize(h):
                        atp = psum.tile([P, P], dt.bfloat16, tag="tr", bufs=1)
                        nc.tensor.transpose(atp[:], anorms[h][:], ident[:])
                        nc.vector.tensor_copy(aq[:, h], atp[:])

                    qk(0)
                    qk(1)
                    av(0)
                    qk(2)
                    av(1)
                    qk(3)
                    av(2)
                    av(3)
                    for h in range(HPG):
                        finalize(h)
                    return aq

                def outproj(qi, aq):
                    for n in range(D // 512):
                        ops = psum.tile([P, 512], dt.float32, tag="q", bufs=3)
                        for f in range(FPG // P):
                            nc.tensor.matmul(
                                ops[:], aq[:, f],
                                wo_sb[:, f, n * 512:(n + 1) * 512],
                                start=(f == 0), stop=(f == FPG // P - 1))
                        osb = work.tile([P, 512], dt.bfloat16, tag="ob",
                                        bufs=4)
                        if n % 2 == 0:
                            nc.vector.tensor_copy(osb[:], ops[:])
                        else:
                            nc.scalar.copy(osb[:], ops[:])
                        nc.sync.dma_start(
                            out[qi * P:(qi + 1) * P, n * 512:(n + 1) * 512],
                            osb[:])

                for h in range(HPG):
                    qproj(0, h)
                aqs = {}
                for qi in range(QT):
                    aqs[qi] = attention(qi)
                    if qi < QT - 4:
                        qproj(qi // 4 + 1, qi % 4)
                    if qi >= 2:
                        outproj(qi - 2, aqs.pop(qi - 2))
                outproj(QT - 2, aqs.pop(QT - 2))
                outproj(QT - 1, aqs.pop(QT - 1))

    nc.compile()
    return nc


def _get_program(causal: bool, has_bias: bool):
    key = (causal, has_bias)
    if key not in _COMPILED:
        _COMPILED[key] = _build(causal, KCH if has_bias else KCH - 1)
    return _COMPILED[key]


def _detect_mask(attention_mask: np.ndarray) -> bool:
    am = np.asarray(attention_mask).reshape(S, S)
    if not am.any():
        return False
    tri = np.tril(np.ones((S, S), dtype=bool))
    if np.all(am[tri] == 0.0) and np.all(am[~tri] <= -1e8):
        return True
    raise ValueError("kernel supports causal (0/-1e9) or all-zero masks only")


def _prep_core_inputs(hidden_states, Wq, bq, Wk, bk, Wv, bv, Wo,
                      alibi_slopes, causal):
    """Build the 8 per-core input maps (host-side shard + fold)."""
    KA = KCH * P
    k = np.arange(P, dtype=np.float64)[:, None]          # k_local
    q = np.arange(P, dtype=np.float64)[None, :]          # q_local
    in_maps = [None] * NCORES
    for b in range(B):
        xTa = np.zeros((KA, S), dtype=_BF16)
        xTa[:D] = np.ascontiguousarray(hidden_states[b].T).astype(_BF16)
        xTa[D] = 1.0
        for g in range(NG):
            sl = np.asarray(alibi_slopes[g * HPG:(g + 1) * HPG], np.float64)
            wqTa = np.zeros((KA, FPG), dtype=_BF16)
            wqTa[:D] = (SCALE * Wq[g * FPG:(g + 1) * FPG, :].T).astype(_BF16)
            wqTa[D] = (SCALE * bq[g * FPG:(g + 1) * FPG]).astype(_BF16)
            wkTa = np.zeros((KA, DH), dtype=np.float32)
            wkTa[:D] = Wk[g * DH:(g + 1) * DH, :].T
            wkTa[D] = bk[g * DH:(g + 1) * DH]
            wvTa = np.zeros((KA, DH), dtype=np.float32)
            wvTa[:D] = Wv[g * DH:(g + 1) * DH, :].T
            wvTa[D] = bv[g * DH:(g + 1) * DH]
            # pack [KCH*P, DH] -> [P, KCH*DH]: wkP[p, c*DH+f] = wkTa[c*P+p, f]
            wkPm = np.ascontiguousarray(
                wkTa.reshape(KCH, P, DH).transpose(1, 0, 2).reshape(
                    P, KCH * DH)).astype(_BF16)
            wvPm = np.ascontiguousarray(
                wvTa.reshape(KCH, P, DH).transpose(1, 0, 2).reshape(
                    P, KCH * DH)).astype(_BF16)
            woT = np.ascontiguousarray(
                Wo[:, g * FPG:(g + 1) * FPG].T).astype(_BF16)
            im = {"xTa": xTa, "wqTa": wqTa, "wkP": wkPm, "wvP": wvPm,
                  "woT": woT}
            if causal:
                # mmf[k, m, h, q]: e^{slope*(k - q - 128*d)}, d = QT-1-m,
                # triangular-masked on the diagonal tile (d=0)
                mmv = np.zeros((P, QT, HPG, P), dtype=np.float64)
                for m in range(QT):
                    d = QT - 1 - m
                    arg = sl[None, :, None] * (k[:, None, :] - q[None, :]
                                               - 128.0 * d)
                    v = np.exp(np.minimum(arg, 0.0))
                    if d == 0:
                        v = v * (k[:, None, :] <= q[None, :])
                    mmv[:, m] = v
                im["mmf"] = mmv.astype(_BF16)
            else:
                t = np.arange(QT, dtype=np.float64)[None, :]
                ev = np.exp(sl[None, None, :]
                            * (k + 128.0 * t[:, :, None] - (S - 1.0)))
                im["evec"] = ev.astype(np.float32)
            in_maps[b * NG + g] = im
    return in_maps


def kernel(hidden_states, attention_mask, Wq, bq, Wk, bk, Wv, bv, Wo, bo,
           alibi_slopes):
    import time
    causal = _detect_mask(attention_mask)
    has_bias = bool(np.asarray(bq).any() or np.asarray(bk).any()
                    or np.asarray(bv).any())
    nc = _get_program(causal, has_bias)
    in_maps = _prep_core_inputs(
        np.asarray(hidden_states, np.float32), np.asarray(Wq, np.float32),
        np.asarray(bq, np.float32), np.asarray(Wk, np.float32),
        np.asarray(bk, np.float32), np.asarray(Wv, np.float32),
        np.asarray(bv, np.float32), np.asarray(Wo, np.float32),
        np.asarray(alibi_slopes, np.float32), causal)
    t0 = time.perf_counter()
    res = run_bass_kernel_spmd(nc, in_maps, list(range(NCORES)))
    t1 = time.perf_counter()
    LAST_RUN["wall_s"] = t1 - t0
    out = np.zeros((B, S, D), dtype=np.float32)
    for b in range(B):
        for g in range(NG):
            out[b] += np.asarray(res.results[b * NG + g]["out"],
                                 dtype=np.float32)
        out[b] += np.asarray(bo, np.float32)[None, :]
    return out


# revision 36
# speedup vs baseline: 2.4548x; 2.4548x over previous
"""Causal ALiBi attention (GQA) on 8 Trainium2 NeuronCores.

Sharding: 2 (batch) x 4 (head groups). Core (b, g) computes batch b, query
heads 4g..4g+3 and kv head g, producing a partial output (its heads'
contribution to the out-projection). Host sums the 4 partials per batch and
adds bo.

Per-core kernel (matmuls bf16, fp32 PSUM accumulation):
  - input DMA is chunked along the contraction dim and K/V projections
    consume chunks as they land, so the PE never waits for the full 9 MB
    x^T transfer. V is produced directly token-major (lhsT = x^T chunk).
  - scores are computed TRANSPOSED: S^T[k,q] = sum_d kT[d,k] qT[d,q], so
    softmax probabilities come out of the PE already k-major and feed the
    AV matmul with no transposes at all.
  - softmax without max-reduction: P''[k,q] = e^{S-12} * M_d[k,q] where
    M_d[k,q] = e^{slope*(k-q-128d)} (d = qi-kt tile diagonal index,
    triangular-masked at d=0) is an exact precomputed bf16 factor. exp is
    one wide ACT instruction per 512-col chunk (scalar bias -12); the
    alibi+mask factor is one batched DVE multiply per chunk. Entries that
    underflow bf16 have true softmax weight < e^-75 of their row max.
  - AV rhs is [V | 1]-augmented, so PSUM column 128 accumulates the
    denominator l_q for free; normalization rides the PSUM->SBUF copy.
  - attention out is PE-transposed to [dh, q]; out-projection contracts
    feature chunks with Wo^T; partial out rows DMA to HBM as bf16.
  - emission interleaves Q-projection chunks, attention tiles and the
    out-projection so PE keeps running while ACT does exp and DVE does
    the mask multiplies.
"""

import numpy as np
import ml_dtypes

import concourse.bass as bass
import concourse.tile as tile
from concourse import bacc, mybir
from concourse.bass_utils import run_bass_kernel_spmd
from concourse.masks import make_identity

B, S, D = 2, 2048, 2048
H, KV, DH = 16, 4, 128
SCALE = 1.0 / np.sqrt(DH)
NCORES = 8
NG = 4            # head groups (= kv heads); one per core within a batch
HPG = H // NG     # query heads per group (4)
FPG = HPG * DH    # features per group (512)
P = 128           # partitions
QT = S // P       # q tiles (16)
KCH = (D + P) // P  # contraction chunks incl. bias ones-row chunk (17)
VW = P + 4        # padded AV psum width (129 used)
EXP_SHIFT = 12.0  # constant exponent headroom margin

_BF16 = ml_dtypes.bfloat16
_COMPILED = {}    # (causal, has_bias) -> compiled Bacc program
LAST_RUN = {}     # timing info from the most recent run


def _build(causal: bool, nkch: int = KCH):
    nc = bacc.Bacc("TRN2", target_bir_lowering=False, debug=False,
                   num_devices=NCORES)
    dt = mybir.dt
    KA = KCH * P

    xTa = nc.dram_tensor("xTa", [KA, S], dt.bfloat16, kind="ExternalInput")
    wqTa = nc.dram_tensor("wqTa", [KA, FPG], dt.bfloat16, kind="ExternalInput")
    # wk/wv packed partition-major: wkP[p, c*DH+f] = Wk^T[c*P+p, f]
    wkP = nc.dram_tensor("wkP", [P, KCH * DH], dt.bfloat16,
                         kind="ExternalInput")
    wvP = nc.dram_tensor("wvP", [P, KCH * DH], dt.bfloat16,
                         kind="ExternalInput")
    woT = nc.dram_tensor("woT", [FPG, D], dt.bfloat16, kind="ExternalInput")
    if causal:
        # mmf[k, m, h, q]: alibi factor for tile diagonal d = QT-1-m
        mmf = nc.dram_tensor("mmf", [P, QT, HPG, P], dt.bfloat16,
                             kind="ExternalInput")
    else:
        # evec[k, t, h]: e^{slope*(k_global - (S-1))} folded into V per head
        evec = nc.dram_tensor("evec", [P, QT, HPG], dt.float32,
                              kind="ExternalInput")
    out = nc.dram_tensor("out", [S, D], dt.bfloat16, kind="ExternalOutput")

    with tile.TileContext(nc) as tc:
        with tc.tile_pool(name="persist", bufs=1) as persist:
            # ---- persistent SBUF tiles ----
            ident = persist.tile([P, P], dt.bfloat16)
            make_identity(nc, ident[:])
            bshift = persist.tile([P, 1], dt.float32)
            nc.vector.memset(bshift[:], -EXP_SHIFT)
            # kT/vsb split into independent tiles so their PSUM->SBUF fills
            # can run on different engines without same-tile write chaining
            kTs = [persist.tile([P, S // 2], dt.bfloat16, name=f"kT{i}")
                   for i in range(2)]
            nv = 1 if causal else HPG
            vsbs = [persist.tile([P, nv, 4, VW], dt.bfloat16, name=f"vsb{i}")
                    for i in range(4)]

            def kT(kt):
                # [P, P] slice for k-tile kt
                return kTs[kt // 8][:, (kt % 8) * P:(kt % 8 + 1) * P]

            def vsl(h, kt):
                return vsbs[kt // 4][:, 0 if causal else h, kt % 4, 0:P + 1]
            qT = persist.tile([P, HPG, S], dt.bfloat16)
            xTa_sb = persist.tile([P, nkch, S], dt.bfloat16)
            wq_sb = persist.tile([P, nkch, FPG], dt.bfloat16)
            wk_sb = persist.tile([P, nkch, DH], dt.bfloat16)
            wv_sb = persist.tile([P, nkch, DH], dt.bfloat16)
            wo_sb = persist.tile([P, FPG // P, D], dt.bfloat16)
            if causal:
                mm_sb = persist.tile([P, QT, HPG, P], dt.bfloat16)
            else:
                ev_sb = persist.tile([P, QT, HPG], dt.float32)

            # ---- DMA issue order: phase-1-critical first; chunk-0 weights
            # land in ~200ns so the first K/V matmuls only wait on x chunk 0
            nc.sync.dma_start(wk_sb[:, 0], wkP[:, 0:DH])
            nc.sync.dma_start(wv_sb[:, 0], wvP[:, 0:DH])
            nc.sync.dma_start(xTa_sb[:, 0], xTa[0:P, :])
            nc.sync.dma_start(wk_sb[:, 1:nkch].rearrange("p c f -> p (c f)"),
                              wkP[:, DH:nkch * DH])
            nc.sync.dma_start(wv_sb[:, 1:nkch].rearrange("p c f -> p (c f)"),
                              wvP[:, DH:nkch * DH])
            for c in range(1, nkch):
                nc.sync.dma_start(xTa_sb[:, c], xTa[c * P:(c + 1) * P, :])
            if causal:
                nc.sync.dma_start(mm_sb[:, 12:16], mmf[:, 12:16])
            else:
                nc.sync.dma_start(ev_sb[:], evec[:])
            for c in range(nkch):
                nc.sync.dma_start(wq_sb[:, c], wqTa[c * P:(c + 1) * P, :])
            if causal:
                nc.sync.dma_start(mm_sb[:, 8:12], mmf[:, 8:12])
            for f in range(FPG // P):
                nc.sync.dma_start(wo_sb[:, f], woT[f * P:(f + 1) * P, :])
            if causal:
                nc.sync.dma_start(mm_sb[:, 4:8], mmf[:, 4:8])
                nc.sync.dma_start(mm_sb[:, 0:4], mmf[:, 0:4])

            # ---- phase 1: K/V projections, chunk-outer (overlaps DMA) ----
            # PSUM allows only ONE open multi-instruction accumulation group
            # per bank (a later start=True zeroes the whole bank), so each
            # V-tile accumulator owns a bank and V runs in 4 sweeps
            with tc.tile_pool(name="psum1", bufs=1, space="PSUM") as psum1:
                psKs = [psum1.tile([P, 2, 512], dt.float32, tag=f"pk{i}",
                                   bufs=1, name=f"psK{i}") for i in range(2)]
                for j in range(4):
                    nc.vector.memset(vsbs[j][:, :, :, P], 1.0)

                def vsweep(s, with_k):
                    pvs = [psum1.tile([P, P], dt.float32, tag=f"pv{j}",
                                      bufs=1, name=f"psV{s}_{j}")
                           for j in range(4)]
                    for c in range(nkch):
                        st, sp = (c == 0), (c == nkch - 1)
                        if with_k:
                            for t4 in range(4):
                                nc.tensor.matmul(
                                    psKs[t4 // 2][:, t4 % 2], wk_sb[:, c],
                                    xTa_sb[:, c, t4 * 512:(t4 + 1) * 512],
                                    start=st, stop=sp)
                        for j in range(4):
                            t = s * 4 + j
                            nc.tensor.matmul(pvs[j],
                                             xTa_sb[:, c, t * P:(t + 1) * P],
                                             wv_sb[:, c], start=st, stop=sp)
                    if with_k:
                        for t4 in range(4):
                            if t4 < 2:
                                nc.vector.tensor_copy(
                                    kTs[0][:, t4 * 512:(t4 + 1) * 512],
                                    psKs[0][:, t4])
                            else:
                                nc.scalar.copy(
                                    kTs[1][:, (t4 % 2) * 512:
                                           (t4 % 2 + 1) * 512],
                                    psKs[1][:, t4 % 2])
                    dst = vsbs[s]
                    eng = (nc.scalar, nc.vector)[s % 2]
                    for j in range(4):
                        if causal:
                            if eng is nc.scalar:
                                nc.scalar.copy(dst[:, 0, j, 0:P], pvs[j])
                            else:
                                nc.vector.tensor_copy(dst[:, 0, j, 0:P],
                                                      pvs[j])
                        else:
                            t = s * 4 + j
                            for h in range(HPG):
                                nc.vector.tensor_scalar_mul(
                                    dst[:, h, j, 0:P], pvs[j],
                                    ev_sb[:, t, h:h + 1])
                                nc.vector.tensor_copy(dst[:, h, j, P:P + 1],
                                                      ev_sb[:, t, h:h + 1])

                for s in range(4):
                    vsweep(s, with_k=(s == 0))

            # ---- phase 2+3: Q-proj / attention / out-proj interleaved ----
            with (
                tc.tile_pool(name="psum", bufs=1, space="PSUM") as psum,
                tc.tile_pool(name="work", bufs=1) as work,
                tc.tile_pool(name="small", bufs=4) as small,
            ):
                def qproj(tq, h):
                    ps = psum.tile([P, 512], dt.float32, tag="q", bufs=3)
                    for c in range(nkch):
                        nc.tensor.matmul(
                            ps[:], wq_sb[:, c, h * P:(h + 1) * P],
                            xTa_sb[:, c, tq * 512:(tq + 1) * 512],
                            start=(c == 0), stop=(c == nkch - 1))
                    nc.scalar.copy(qT[:, h, tq * 512:(tq + 1) * 512], ps[:])

                def attention(qi, filler=None):
                    nkt = qi + 1 if causal else QT
                    nch = (nkt + 3) // 4
                    aq = work.tile([P, HPG, P], dt.bfloat16, tag="aq", bufs=4)
                    pex = [[None] * nch for _ in range(HPG)]

                    def qk(h):
                        for c in range(nch):
                            w = min(4, nkt - c * 4)
                            pexp = work.tile([P, 512], dt.bfloat16, tag="pex",
                                             bufs=12)
                            pex[h][c] = pexp
                            ss = psum.tile([P, 512], dt.float32, tag="sc",
                                           bufs=2)
                            for j in range(w):
                                nc.tensor.matmul(
                                    ss[:, j * P:(j + 1) * P],
                                    kT(c * 4 + j),
                                    qT[:, h, qi * P:(qi + 1) * P],
                                    start=True, stop=True)
                            nc.scalar.activation(
                                pexp[:, 0:w * P], ss[:, 0:w * P],
                                mybir.ActivationFunctionType.Exp,
                                bias=bshift[:], scale=1.0)
                            if causal:
                                seg = pexp[:, 0:w * P].rearrange(
                                    "p (n q) -> p n q", q=P)
                                m0 = QT - 1 - qi + c * 4
                                nc.vector.tensor_mul(seg, seg,
                                                     mm_sb[:, m0:m0 + w, h])

                    anorms = [None] * HPG

                    def av(h):
                        avt = psum.tile([P, VW], dt.float32, tag="av", bufs=2)
                        vh = 0 if causal else h
                        for kt in range(nkt):
                            nc.tensor.matmul(
                                avt[:, 0:P + 1],
                                pex[h][kt // 4][:, (kt % 4) * P:
                                                (kt % 4) * P + P],
                                vsl(vh, kt),
                                start=(kt == 0), stop=(kt == nkt - 1))
                        rec = small.tile([P, 1], dt.float32, tag="rec")
                        nc.vector.reciprocal(rec[:], avt[:, P:P + 1])
                        anorm = small.tile([P, P], dt.bfloat16, tag="an",
                                           bufs=8)
                        nc.vector.tensor_scalar_mul(anorm[:], avt[:, 0:P],
                                                    rec[:])
                        anorms[h] = anorm

                    def finalize(h):
                        atp = psum.tile([P, P], dt.bfloat16, tag="tr", bufs=1)
                        nc.tensor.transpose(atp[:], anorms[h][:], ident[:])
                        nc.vector.tensor_copy(aq[:, h], atp[:])

                    qk(0)
                    qk(1)
                    av(0)
                    qk(2)
                    av(1)
                    qk(3)
                    av(2)
                    if filler:
                        filler()
                    av(3)
                    for h in range(HPG):
                        finalize(h)
                    return aq

                def outproj(qi, aq):
                    for n in range(D // 512):
                        ops = psum.tile([P, 512], dt.float32, tag="q", bufs=3)
                        for f in range(FPG // P):
                            nc.tensor.matmul(
                                ops[:], aq[:, f],
                                wo_sb[:, f, n * 512:(n + 1) * 512],
                                start=(f == 0), stop=(f == FPG // P - 1))
                        osb = work.tile([P, 512], dt.bfloat16, tag="ob",
                                        bufs=4)
                        if n % 2 == 0:
                            nc.vector.tensor_copy(osb[:], ops[:])
                        else:
                            nc.scalar.copy(osb[:], ops[:])
                        nc.sync.dma_start(
                            out[qi * P:(qi + 1) * P, n * 512:(n + 1) * 512],
                            osb[:])

                for h in range(HPG):
                    qproj(0, h)
                aqs = {}
                for qi in range(QT):
                    fill = (lambda q=qi: outproj(q - 2, aqs.pop(q - 2))) \
                        if qi >= 2 else None
                    aqs[qi] = attention(qi, fill)
                    if qi < QT - 4:
                        qproj(qi // 4 + 1, qi % 4)
                outproj(QT - 2, aqs.pop(QT - 2))
                outproj(QT - 1, aqs.pop(QT - 1))

    nc.compile()
    return nc


def _get_program(causal: bool, has_bias: bool):
    key = (causal, has_bias)
    if key not in _COMPILED:
        _COMPILED[key] = _build(causal, KCH if has_bias else KCH - 1)
    return _COMPILED[key]


def _detect_mask(attention_mask: np.ndarray) -> bool:
    am = np.asarray(attention_mask).reshape(S, S)
    if not am.any():
        return False
    tri = np.tril(np.ones((S, S), dtype=bool))
    if np.all(am[tri] == 0.0) and np.all(am[~tri] <= -1e8):
        return True
    raise ValueError("kernel supports causal (0/-1e9) or all-zero masks only")


def _prep_core_inputs(hidden_states, Wq, bq, Wk, bk, Wv, bv, Wo,
                      alibi_slopes, causal):
    """Build the 8 per-core input maps (host-side shard + fold)."""
    KA = KCH * P
    k = np.arange(P, dtype=np.float64)[:, None]          # k_local
    q = np.arange(P, dtype=np.float64)[None, :]          # q_local
    in_maps = [None] * NCORES
    for b in range(B):
        xTa = np.zeros((KA, S), dtype=_BF16)
        xTa[:D] = np.ascontiguousarray(hidden_states[b].T).astype(_BF16)
        xTa[D] = 1.0
        for g in range(NG):
            sl = np.asarray(alibi_slopes[g * HPG:(g + 1) * HPG], np.float64)
            wqTa = np.zeros((KA, FPG), dtype=_BF16)
            wqTa[:D] = (SCALE * Wq[g * FPG:(g + 1) * FPG, :].T).astype(_BF16)
            wqTa[D] = (SCALE * bq[g * FPG:(g + 1) * FPG]).astype(_BF16)
            wkTa = np.zeros((KA, DH), dtype=np.float32)
            wkTa[:D] = Wk[g * DH:(g + 1) * DH, :].T
            wkTa[D] = bk[g * DH:(g + 1) * DH]
            wvTa = np.zeros((KA, DH), dtype=np.float32)
            wvTa[:D] = Wv[g * DH:(g + 1) * DH, :].T
            wvTa[D] = bv[g * DH:(g + 1) * DH]
            # pack [KCH*P, DH] -> [P, KCH*DH]: wkP[p, c*DH+f] = wkTa[c*P+p, f]
            wkPm = np.ascontiguousarray(
                wkTa.reshape(KCH, P, DH).transpose(1, 0, 2).reshape(
                    P, KCH * DH)).astype(_BF16)
            wvPm = np.ascontiguousarray(
                wvTa.reshape(KCH, P, DH).transpose(1, 0, 2).reshape(
                    P, KCH * DH)).astype(_BF16)
            woT = np.ascontiguousarray(
                Wo[:, g * FPG:(g + 1) * FPG].T).astype(_BF16)
            im = {"xTa": xTa, "wqTa": wqTa, "wkP": wkPm, "wvP": wvPm,
                  "woT": woT}
            if causal:
                # mmf[k, m, h, q]: e^{slope*(k - q - 128*d)}, d = QT-1-m,
                # triangular-masked on the diagonal tile (d=0)
                mmv = np.zeros((P, QT, HPG, P), dtype=np.float64)
                for m in range(QT):
                    d = QT - 1 - m
                    arg = sl[None, :, None] * (k[:, None, :] - q[None, :]
                                               - 128.0 * d)
                    v = np.exp(np.minimum(arg, 0.0))
                    if d == 0:
                        v = v * (k[:, None, :] <= q[None, :])
                    mmv[:, m] = v
                im["mmf"] = mmv.astype(_BF16)
            else:
                kc = np.arange(P, dtype=np.float64)[:, None, None]
                tc_ = np.arange(QT, dtype=np.float64)[None, :, None]
                ev = np.exp(sl[None, None, :]
                            * (kc + 128.0 * tc_ - (S - 1.0)))
                im["evec"] = ev.astype(np.float32)
            in_maps[b * NG + g] = im
    return in_maps


def kernel(hidden_states, attention_mask, Wq, bq, Wk, bk, Wv, bv, Wo, bo,
           alibi_slopes):
    import time
    causal = _detect_mask(attention_mask)
    has_bias = bool(np.asarray(bq).any() or np.asarray(bk).any()
                    or np.asarray(bv).any())
    nc = _get_program(causal, has_bias)
    in_maps = _prep_core_inputs(
        np.asarray(hidden_states, np.float32), np.asarray(Wq, np.float32),
        np.asarray(bq, np.float32), np.asarray(Wk, np.float32),
        np.asarray(bk, np.float32), np.asarray(Wv, np.float32),
        np.asarray(bv, np.float32), np.asarray(Wo, np.float32),
        np.asarray(alibi_slopes, np.float32), causal)
    t0 = time.perf_counter()
    res = run_bass_kernel_spmd(nc, in_maps, list(range(NCORES)))
    t1 = time.perf_counter()
    LAST_RUN["wall_s"] = t1 - t0
    out = np.zeros((B, S, D), dtype=np.float32)
    for b in range(B):
        for g in range(NG):
            out[b] += np.asarray(res.results[b * NG + g]["out"],
                                 dtype=np.float32)
        out[b] += np.asarray(bo, np.float32)[None, :]
    return out
